# revision 29
# baseline (speedup 1.0000x reference)
"""AConvCircular3D kernel for 8 trn2 NeuronCores.

Sharding: core i handles (batch b = i//4, head h = i%4).
Per core, for its (b, h):
  - 3x3x3 circular conv of x[b] -> 32 channels [q(8) k(8) v(8) init(8)]
    (q-scale folded into weights; K=96 contraction via 3 dx-shifted
    copies of padded x; f32r matmuls)
  - softmax attention for head h (N=4096, dk=dv=8), no max-subtraction:
      for each n-quarter: for each key-tile t:
        ST = K_t^T Q (f32r)  -> exp on ScalarE -> ET (bf16)
        AV^T[v, n] += V'_t @ ET   (V' = [V; ones] stationary, 9 cols)
      denominator comes from the ones row; normalize after transpose.
  - torch-faithful reshape (n-major A) via DRAM bounce, partial 1x1 conv
Host sums the 1x1 partials over the 4 head-cores per batch, adds bias,
and concatenates with the gathered init channels.
"""
import os
import sys

for _p in ("/opt/trn_rl_repo", "/root/.axon_site/_ro/trn_rl_repo"):
    if os.path.isdir(_p) and _p not in sys.path:
        sys.path.insert(0, _p)

import numpy as np
import ml_dtypes

NUM_HEADS = 4
DKH = 8
DVH = 8
IN_CH = 32
S = 16
NSP = S * S * S           # 4096 spatial positions
PADW = S + 2              # 18
PADV = PADW ** 3          # 5832
M_TILE = 128
N_MTILES = NSP // M_TILE  # 32

_CACHE = {}


LDW_OPT = False


def _patch_ldw_opt():
    import concourse.bass_utils as bu
    if getattr(bu, "_ldw_patched", False):
        return
    orig = bu.run_command

    def run_command_ldw(cmd, *a, **kw):
        if isinstance(cmd, list):
            cmd = ["--enable-ldw-opt=true" if c == "--enable-ldw-opt=false" else c
                   for c in cmd]
        return orig(cmd, *a, **kw)

    bu.run_command = run_command_ldw
    bu._ldw_patched = True


def _build(n_cores=8):
    import concourse.bass as bass
    import concourse.mybir as mybir
    import concourse.tile as tile
    from concourse.tile import add_dep_helper
    from concourse import bacc
    from concourse.bass import ts
    from concourse.masks import make_identity

    BF16 = mybir.dt.bfloat16
    F32 = mybir.dt.float32
    F32R = mybir.dt.float32r
    EXP = mybir.ActivationFunctionType.Exp
    COPY = mybir.ActivationFunctionType.Copy

    if LDW_OPT:
        _patch_ldw_opt()
    nc = bacc.Bacc("TRN2", target_bir_lowering=False, debug=False,
                   num_devices=n_cores)

    xpad = nc.declare_dram_parameter("xpad", [IN_CH, PADV], BF16, isOutput=False)
    wcv = nc.declare_dram_parameter("wcv", [96, 9 * 32], BF16, isOutput=False)
    wout = nc.declare_dram_parameter("wout", [8, 32], F32, isOutput=False)
    selp = nc.declare_dram_parameter("sel", [128, 16], F32, isOutput=False)
    iout = nc.declare_dram_parameter("iout", [8, NSP], BF16, isOutput=True)
    pout = nc.declare_dram_parameter("pout", [32, NSP], F32, isOutput=True)

    with tile.TileContext(nc) as tc:
        with tc.tile_pool(name="sb", bufs=1) as sb, \
             tc.tile_pool(name="et", bufs=3) as etp, \
             tc.tile_pool(name="dr", bufs=1, space="DRAM") as drp:
            a_bounce = drp.tile([NSP, DVH], F32)

            # ---- stage padded x, replicated 3x with dx shifts ----
            xp = sb.tile([96, PADV], BF16)
            for g in range(3):
                n = PADV - g
                for piece in range(8):
                    lo = (n * piece) // 8
                    hi = (n * (piece + 1)) // 8
                    nc.sync.dma_start(out=xp[32 * g:32 * g + 32, lo:hi],
                                      in_=xpad[:, g + lo:g + hi])
            xp4 = xp[:].rearrange("p (z y x) -> p z y x", z=PADW, y=PADW, x=PADW)

            w_sb = sb.tile([96, 9 * 32], BF16)
            nc.sync.dma_start(out=w_sb[:], in_=wcv[:])
            wout_f = sb.tile([8, 32], F32)
            nc.sync.dma_start(out=wout_f[:], in_=wout[:])
            wout_b = sb.tile([8, 32], BF16)
            nc.vector.tensor_copy(out=wout_b[:], in_=wout_f[:])
            ident = sb.tile([128, 128], BF16)
            make_identity(nc, ident)

            stg16 = sb.tile([32, NSP], BF16)
            q_rep = sb.tile([72, NSP], BF16)
            k_rep = sb.tile([72, NSP], BF16)
            v_sb = sb.tile([9, NSP], BF16)
            nc.vector.memset(v_sb[:], 1.0)
            sel_sb = sb.tile([128, 16], F32)
            nc.sync.dma_start(out=sel_sb[:], in_=selp[:])

            # ---- conv: 9 rounds (dz,dy) x 8 chunks (z pairs), K=96, f32r ----
            # evictions + K/V remap chunked so attention can start early
            with tc.tile_pool(name="cv", bufs=1, space="PSUM") as cvp:
                cv = cvp.tile([32, NSP], F32)
                for c in range(8):
                    for r in range(9):
                        dz, dy = r // 3, r % 3
                        nc.tensor.matmul(cv[:, ts(c, 512)],
                                         w_sb[:, ts(r, 32)],
                                         xp4[:, 2 * c + dz:2 * c + dz + 2,
                                             dy:dy + S, 0:S],
                                         start=(r == 0), stop=(r == 8))
                    nc.vector.tensor_copy(out=stg16[:, ts(c, 512)],
                                          in_=cv[:, ts(c, 512)])
                    for r in range(3):
                        nc.sync.dma_start(out=q_rep[32 * r:32 * r + 8, ts(c, 512)],
                                          in_=stg16[0:8, ts(c, 512)])
                        nc.sync.dma_start(out=k_rep[32 * r:32 * r + 8, ts(c, 512)],
                                          in_=stg16[8:16, ts(c, 512)])
                    nc.sync.dma_start(out=v_sb[0:8, ts(c, 512)],
                                      in_=stg16[16:24, ts(c, 512)])
                    nc.sync.dma_start(out=iout[:, ts(c, 512)],
                                      in_=stg16[24:32, ts(c, 512)])


            # ---- VT' tiles: transpose V' [9,128] chunks -> vt[128, t, 9] ----
            vt = sb.tile([128, N_MTILES, 9], BF16)
            with tc.tile_pool(name="tp", bufs=2, space="PSUM") as tpp:
                for t in range(N_MTILES):
                    tp = tpp.tile([128, 9], F32)
                    nc.tensor.matmul(tp[:], v_sb[:, ts(t, M_TILE)],
                                     ident[0:9, 0:9], start=True, stop=True)
                    nc.vector.tensor_copy(out=vt[:, t, :], in_=tp[:])

            # ---- attention: n-512-chunk outer, m-groups of 3 inner ----
            # 3-way row-tiled logits (bands 0/32/64 of q_rep/k_rep) and
            # 3-way col-tiled AV partials (av bands 0/32/64); the ones row
            # of V' gives the softmax denominator. Band partials are summed
            # (and transposed to [n, 9]) by a selector matmul afterwards.
            groups = [(3 * g, min(3, N_MTILES - 3 * g))
                      for g in range((N_MTILES + 2) // 3)]
            av_cat = sb.tile([128, 8, 512], F32)
            zc = sb.tile([1, 128], BF16)
            zr = sb.tile([1, 512], BF16)
            nc.vector.memset(zc[:], 0.0)
            nc.vector.memset(zr[:], 0.0)
            with tc.tile_pool(name="av", bufs=2, space="PSUM") as avp, \
                 tc.tile_pool(name="lg", bufs=2, space="PSUM") as lgp:
                for ch in range(8):
                    av_ch = avp.tile([128, 512], F32)
                    # zero the whole accumulator (incl. unused partition rows,
                    # which would otherwise contain stale psum NaNs) and set
                    # has_written bank-wide; creates ordering dep for all bands
                    nc.tensor.matmul(av_ch[:], zc[:], zr[:], start=True,
                                     stop=False)
                    for gi, (t0, gn) in enumerate(groups):
                        lg = lgp.tile([128, 512 * gn], F32, tag="lg")
                        for r in range(gn):
                            nc.tensor.matmul(
                                lg[:, ts(r, 512)],
                                k_rep[32 * r:32 * r + 8, ts(t0 + r, M_TILE)],
                                q_rep[32 * r:32 * r + 8, ts(ch, 512)],
                                start=True, stop=True,
                                tile_position=(32 * r, 0))
                        et = etp.tile([128, 512 * gn], BF16, tag="et")
                        nc.scalar.activation(out=et[:], in_=lg[:], func=EXP)
                        for r in range(gn):
                            nc.tensor.matmul(
                                av_ch[32 * r:32 * r + 9, :],
                                vt[:, t0 + r, :],
                                et[:, ts(r, 512)],
                                start=False, stop=(gi == len(groups) - 1),
                                tile_position=(0, 32 * r))
                    nc.vector.tensor_copy(out=av_cat[:, ch, :], in_=av_ch[:])

            # ---- band-reduce + transpose via selector matmul ----
            # avf[n128, v] = sum_p av_cat[p, n] * sel[p, v]
            avf = sb.tile([128, N_MTILES, 9], F32)
            with tc.tile_pool(name="tq", bufs=2, space="PSUM") as tqp:
                for t in range(N_MTILES):
                    ch, off = t // 4, (t % 4) * 128
                    tq = tqp.tile([128, 9], F32)
                    nc.tensor.matmul(tq[:],
                                     av_cat[:, ch, off:off + M_TILE],
                                     sel_sb[:, 0:9], start=True, stop=True)
                    nc.vector.tensor_copy(out=avf[:, t, :], in_=tq[:])
            rcp = sb.tile([128, N_MTILES], F32)
            nc.vector.reciprocal(out=rcp[:], in_=avf[:, :, 8])
            a_sb = sb.tile([128, N_MTILES, 8], F32)
            nc.vector.tensor_mul(a_sb[:], avf[:, :, 0:8],
                                 rcp[:].to_broadcast([128, N_MTILES, 8]))

            # ---- funky reshape via DRAM bounce (f32: dma reads of high
            #      bf16 sbuf tiles at 128 partitions corrupt p>=64) ----
            wdma = nc.sync.dma_start(
                out=a_bounce[:].rearrange("(c p) v -> p c v", p=128),
                in_=a_sb[:])
            comb = sb.tile([8, NSP], BF16)
            rdma = nc.gpsimd.dma_start(
                out=comb[:],
                in_=a_bounce[:].rearrange("(c r) v -> c (r v)", c=8))
            add_dep_helper(rdma.ins, wdma.ins,
                           reason="a_bounce dram RAW: read-back after write")

            # ---- partial 1x1 conv ----
            po_sb = sb.tile([32, NSP], F32)
            with tc.tile_pool(name="po", bufs=1, space="PSUM") as pop:
                po = pop.tile([32, NSP], F32)
                for c in range(8):
                    nc.tensor.matmul(po[:, ts(c, 512)], wout_b[:],
                                     comb[:, ts(c, 512)], start=True, stop=True)
                    nc.scalar.activation(out=po_sb[:, ts(c, 512)],
                                         in_=po[:, ts(c, 512)], func=COPY)
                    nc.sync.dma_start(out=pout[:, ts(c, 512)],
                                      in_=po_sb[:, ts(c, 512)])

    nc.compile()
    return nc


def _get_nc(n_cores=8):
    if n_cores not in _CACHE:
        _CACHE[n_cores] = _build(n_cores)
    return _CACHE[n_cores]


def _host_inputs(x, w_init, w_qkv, w_out):
    """Build per-core input maps. Core i = (b = i//4, h = i%4)."""
    x = np.asarray(x, dtype=np.float32)
    w_init = np.asarray(w_init, dtype=np.float32)
    w_qkv = np.asarray(w_qkv, dtype=np.float32)
    w_out = np.asarray(w_out, dtype=np.float32)

    xpads = []
    for b in range(2):
        xp = np.pad(x[b], ((0, 0), (1, 1), (1, 1), (1, 1)), mode="wrap")
        xpads.append(np.ascontiguousarray(xp.reshape(IN_CH, PADV)))

    in_maps = []
    scale = DKH ** -0.5
    for i in range(8):
        b, h = i // 4, i % 4
        W = np.concatenate([
            w_qkv[8 * h:8 * h + 8] * scale,
            w_qkv[32 + 8 * h:32 + 8 * h + 8],
            w_qkv[64 + 8 * h:64 + 8 * h + 8],
            w_init[8 * h:8 * h + 8],
        ], axis=0)  # (32 oc, 32 ic, 3, 3, 3)
        wcv = np.zeros((96, 9, 32), dtype=np.float32)
        for g in range(3):
            for dz in range(3):
                for dy in range(3):
                    wcv[32 * g:32 * g + 32, 3 * dz + dy, :] = W[:, :, dz, dy, g].T
        wout_h = np.ascontiguousarray(w_out[:, 8 * h:8 * h + 8, 0, 0, 0].T)
        sel = np.zeros((128, 16), dtype=np.float32)
        for c in range(3):
            for v in range(9):
                sel[32 * c + v, v] = 1.0
        in_maps.append({
            "sel": sel,
            "xpad": xpads[b].astype(ml_dtypes.bfloat16),
            "wcv": np.ascontiguousarray(wcv.reshape(96, 9 * 32)).astype(ml_dtypes.bfloat16),
            "wout": wout_h.astype(np.float32),
        })
    return in_maps


def kernel(x, w_init, w_qkv, w_out, b_out):
    from concourse.bass_utils import run_bass_kernel_spmd

    nc = _get_nc()
    in_maps = _host_inputs(x, w_init, w_qkv, w_out)
    res = run_bass_kernel_spmd(nc, in_maps, core_ids=list(range(8)))

    b_out = np.asarray(b_out, dtype=np.float32)
    out = np.zeros((2, 64, NSP), dtype=np.float32)
    for i in range(8):
        b, h = i // 4, i % 4
        out[b, 8 * h:8 * h + 8] = np.asarray(res.results[i]["iout"], dtype=np.float32)
        out[b, 32:64] += res.results[i]["pout"]
    out[:, 32:64] += b_out[None, :, None]
    return out.reshape(2, 64, S, S, S)


# revision 30
# speedup vs baseline: 1.3441x; 1.3441x over previous
"""AConvCircular3D kernel for 8 trn2 NeuronCores.

Sharding: core i handles (batch b = i//4, head h = i%4).
Per core, for its (b, h):
  - 3x3x3 circular conv of x[b] -> 32 channels [q(8) k(8) v(8) init(8)]
    (q-scale folded into weights; K=96 contraction via 3 dx-shifted
    copies of padded x; f32r matmuls)
  - softmax attention for head h (N=4096, dk=dv=8), no max-subtraction:
      for each n-quarter: for each key-tile t:
        ST = K_t^T Q (f32r)  -> exp on ScalarE -> ET (bf16)
        AV^T[v, n] += V'_t @ ET   (V' = [V; ones] stationary, 9 cols)
      denominator comes from the ones row; normalize after transpose.
  - torch-faithful reshape (n-major A) via DRAM bounce, partial 1x1 conv
Host sums the 1x1 partials over the 4 head-cores per batch, adds bias,
and concatenates with the gathered init channels.
"""
import os
import sys

for _p in ("/opt/trn_rl_repo", "/root/.axon_site/_ro/trn_rl_repo"):
    if os.path.isdir(_p) and _p not in sys.path:
        sys.path.insert(0, _p)

import numpy as np
import ml_dtypes

NUM_HEADS = 4
DKH = 8
DVH = 8
IN_CH = 32
S = 16
NSP = S * S * S           # 4096 spatial positions
PADW = S + 2              # 18
PADV = PADW ** 3          # 5832
M_TILE = 128
N_MTILES = NSP // M_TILE  # 32

_CACHE = {}


LDW_OPT = False


def _patch_ldw_opt():
    import concourse.bass_utils as bu
    if getattr(bu, "_ldw_patched", False):
        return
    orig = bu.run_command

    def run_command_ldw(cmd, *a, **kw):
        if isinstance(cmd, list):
            cmd = ["--enable-ldw-opt=true" if c == "--enable-ldw-opt=false" else c
                   for c in cmd]
        return orig(cmd, *a, **kw)

    bu.run_command = run_command_ldw
    bu._ldw_patched = True


def _build(n_cores=8):
    import concourse.bass as bass
    import concourse.mybir as mybir
    import concourse.tile as tile
    from concourse.tile import add_dep_helper
    from concourse import bacc
    from concourse.bass import ts
    from concourse.masks import make_identity

    BF16 = mybir.dt.bfloat16
    F32 = mybir.dt.float32
    F32R = mybir.dt.float32r
    EXP = mybir.ActivationFunctionType.Exp
    COPY = mybir.ActivationFunctionType.Copy

    if LDW_OPT:
        _patch_ldw_opt()
    nc = bacc.Bacc("TRN2", target_bir_lowering=False, debug=False,
                   num_devices=n_cores)

    xpad = nc.declare_dram_parameter("xpad", [IN_CH, PADV], BF16, isOutput=False)
    wcv = nc.declare_dram_parameter("wcv", [96, 9 * 32], BF16, isOutput=False)
    wout = nc.declare_dram_parameter("wout", [8, 32], F32, isOutput=False)
    selp = nc.declare_dram_parameter("sel", [128, 16], F32, isOutput=False)
    iout = nc.declare_dram_parameter("iout", [8, NSP], BF16, isOutput=True)
    pout = nc.declare_dram_parameter("pout", [32, NSP], F32, isOutput=True)

    with tile.TileContext(nc) as tc:
        with tc.tile_pool(name="sb", bufs=1) as sb, \
             tc.tile_pool(name="et", bufs=3) as etp, \
             tc.tile_pool(name="dr", bufs=1, space="DRAM") as drp:
            a_bounce = drp.tile([NSP, DVH], F32)

            # ---- stage padded x: 4 overlapping z-slabs, replicated 3x with
            #      dx shifts, so conv chunk c only waits for slab c//2 ----
            ZSL = 6 * PADW * PADW           # slab = 6 z-slices
            slabs = []
            for sl in range(4):
                z0 = 4 * sl
                xs = sb.tile([96, ZSL], BF16, tag=f"xs{sl}")
                base = z0 * PADW * PADW
                for g in range(3):
                    lo = base + g
                    hi = min(lo + ZSL, PADV)
                    eng = nc.sync if (sl + g) % 2 == 0 else nc.gpsimd
                    eng.dma_start(out=xs[32 * g:32 * g + 32, 0:hi - lo],
                                  in_=xpad[:, lo:hi])
                slabs.append(xs[:].rearrange("p (z y x) -> p z y x",
                                             z=6, y=PADW, x=PADW))

            w_sb = sb.tile([96, 9 * 32], BF16)
            nc.sync.dma_start(out=w_sb[:], in_=wcv[:])
            wout_f = sb.tile([8, 32], F32)
            nc.sync.dma_start(out=wout_f[:], in_=wout[:])
            wout_b = sb.tile([8, 32], BF16)
            nc.vector.tensor_copy(out=wout_b[:], in_=wout_f[:])
            ident = sb.tile([128, 128], BF16)
            make_identity(nc, ident)

            stg16 = sb.tile([32, NSP], BF16)
            q_rep = sb.tile([72, NSP], BF16)
            k_rep = sb.tile([72, NSP], BF16)
            v_sb = sb.tile([9, NSP], BF16)
            nc.vector.memset(v_sb[:], 1.0)
            sel_sb = sb.tile([128, 16], F32)
            nc.sync.dma_start(out=sel_sb[:], in_=selp[:])

            # ---- conv: 9 rounds (dz,dy) x 8 chunks (z pairs), K=96, f32r ----
            # evictions + K/V remap chunked so attention can start early
            with tc.tile_pool(name="cv", bufs=1, space="PSUM") as cvp:
                cv = cvp.tile([32, NSP], F32)
                for cp in range(4):
                    for r in range(9):
                        dz, dy = r // 3, r % 3
                        for c in (2 * cp, 2 * cp + 1):
                            sl = c // 2
                            zb = 2 * c - 4 * sl
                            nc.tensor.matmul(cv[:, ts(c, 512)],
                                             w_sb[:, ts(r, 32)],
                                             slabs[sl][:, zb + dz:zb + dz + 2,
                                                       dy:dy + S, 0:S],
                                             start=(r == 0), stop=(r == 8))
                    w0 = cp * 1024
                    nc.vector.tensor_copy(out=stg16[:, w0:w0 + 1024],
                                          in_=cv[:, w0:w0 + 1024])
                    for r in range(3):
                        nc.sync.dma_start(
                            out=q_rep[32 * r:32 * r + 8, w0:w0 + 1024],
                            in_=stg16[0:8, w0:w0 + 1024])
                        nc.sync.dma_start(
                            out=k_rep[32 * r:32 * r + 8, w0:w0 + 1024],
                            in_=stg16[8:16, w0:w0 + 1024])
                    nc.gpsimd.dma_start(out=v_sb[0:8, w0:w0 + 1024],
                                        in_=stg16[16:24, w0:w0 + 1024])
                    nc.gpsimd.dma_start(out=iout[:, w0:w0 + 1024],
                                        in_=stg16[24:32, w0:w0 + 1024])


            # ---- VT' tiles: transpose V' [9,128] chunks -> vt[128, t, 9] ----
            vt = sb.tile([128, N_MTILES, 9], BF16)
            with tc.tile_pool(name="tp", bufs=2, space="PSUM") as tpp:
                for t in range(N_MTILES):
                    tp = tpp.tile([128, 9], F32)
                    nc.tensor.matmul(tp[:], v_sb[:, ts(t, M_TILE)],
                                     ident[0:9, 0:9], start=True, stop=True)
                    nc.vector.tensor_copy(out=vt[:, t, :], in_=tp[:])

            # ---- attention: n-512-chunk outer, m-groups of 3 inner ----
            # 3-way row-tiled logits (bands 0/32/64 of q_rep/k_rep) and
            # 3-way col-tiled AV partials (av bands 0/32/64); the ones row
            # of V' gives the softmax denominator. Band partials are summed
            # (and transposed to [n, 9]) by a selector matmul afterwards.
            groups = [(3 * g, min(3, N_MTILES - 3 * g))
                      for g in range((N_MTILES + 2) // 3)]
            av_cat = sb.tile([128, 8, 512], F32)
            zc = sb.tile([1, 128], BF16)
            zr = sb.tile([1, 512], BF16)
            nc.vector.memset(zc[:], 0.0)
            nc.vector.memset(zr[:], 0.0)
            with tc.tile_pool(name="av", bufs=2, space="PSUM") as avp, \
                 tc.tile_pool(name="lg", bufs=2, space="PSUM") as lgp:
                for ch in range(8):
                    av_ch = avp.tile([128, 512], F32)
                    # zero the whole accumulator (incl. unused partition rows,
                    # which would otherwise contain stale psum NaNs) and set
                    # has_written bank-wide; creates ordering dep for all bands
                    nc.tensor.matmul(av_ch[:], zc[:], zr[:], start=True,
                                     stop=False)
                    for gi, (t0, gn) in enumerate(groups):
                        lg = lgp.tile([128, 512 * gn], F32, tag="lg")
                        for r in range(gn):
                            nc.tensor.matmul(
                                lg[:, ts(r, 512)],
                                k_rep[32 * r:32 * r + 8, ts(t0 + r, M_TILE)],
                                q_rep[32 * r:32 * r + 8, ts(ch, 512)],
                                start=True, stop=True,
                                tile_position=(32 * r, 0))
                        et = etp.tile([128, 512 * gn], BF16, tag="et")
                        nc.scalar.activation(out=et[:], in_=lg[:], func=EXP)
                        for r in range(gn):
                            nc.tensor.matmul(
                                av_ch[32 * r:32 * r + 9, :],
                                vt[:, t0 + r, :],
                                et[:, ts(r, 512)],
                                start=False, stop=(gi == len(groups) - 1),
                                tile_position=(0, 32 * r))
                    nc.vector.tensor_copy(out=av_cat[:, ch, :], in_=av_ch[:])

            # ---- band-reduce + transpose via selector matmul ----
            # avf[n128, v] = sum_p av_cat[p, n] * sel[p, v]
            avf = sb.tile([128, N_MTILES, 9], F32)
            with tc.tile_pool(name="tq", bufs=2, space="PSUM") as tqp:
                for t in range(N_MTILES):
                    ch, off = t // 4, (t % 4) * 128
                    tq = tqp.tile([128, 9], F32)
                    nc.tensor.matmul(tq[:],
                                     av_cat[:, ch, off:off + M_TILE],
                                     sel_sb[:, 0:9], start=True, stop=True)
                    nc.vector.tensor_copy(out=avf[:, t, :], in_=tq[:])
            rcp = sb.tile([128, N_MTILES], F32)
            nc.vector.reciprocal(out=rcp[:], in_=avf[:, :, 8])
            a_sb = sb.tile([128, N_MTILES, 8], F32)
            nc.vector.tensor_mul(a_sb[:], avf[:, :, 0:8],
                                 rcp[:].to_broadcast([128, N_MTILES, 8]))

            # ---- funky reshape via DRAM bounce (f32: dma reads of high
            #      bf16 sbuf tiles at 128 partitions corrupt p>=64) ----
            # split into halves over the low n-bits so comb half 0 and the
            # first 1x1 matmuls can start while half 1 is still in flight
            ab5 = a_bounce[:].rearrange("(a b c p) v -> p a b c v",
                                         a=8, b=2, c=2, p=128)
            a5 = a_sb[:].rearrange("p (a b c) v -> p a b c v", b=2, c=2)
            comb = sb.tile([8, NSP], BF16)
            rdmas = []
            for hh in range(2):
                wds = [nc.sync.dma_start(out=ab5[:, :, hh, cc, :],
                                         in_=a5[:, :, hh, cc, :])
                       for cc in range(2)]
                rd = nc.gpsimd.dma_start(
                    out=comb[:, ts(hh, 2048)],
                    in_=a_bounce[:].rearrange("(c hh2 r) v -> c hh2 (r v)",
                                              c=8, hh2=2)[:, hh, :])
                for wd in wds:
                    add_dep_helper(rd.ins, wd.ins,
                                   reason="a_bounce dram RAW half")
                rdmas.append(rd)

            # ---- partial 1x1 conv ----
            po_sb = sb.tile([32, NSP], F32)
            with tc.tile_pool(name="po", bufs=1, space="PSUM") as pop:
                po = pop.tile([32, NSP], F32)
                for c in range(8):
                    nc.tensor.matmul(po[:, ts(c, 512)], wout_b[:],
                                     comb[:, ts(c, 512)], start=True, stop=True)
                    nc.scalar.activation(out=po_sb[:, ts(c, 512)],
                                         in_=po[:, ts(c, 512)], func=COPY)
                    nc.sync.dma_start(out=pout[:, ts(c, 512)],
                                      in_=po_sb[:, ts(c, 512)])

    nc.compile()
    return nc


def _get_nc(n_cores=8):
    if n_cores not in _CACHE:
        _CACHE[n_cores] = _build(n_cores)
    return _CACHE[n_cores]


def _host_inputs(x, w_init, w_qkv, w_out):
    """Build per-core input maps. Core i = (b = i//4, h = i%4)."""
    x = np.asarray(x, dtype=np.float32)
    w_init = np.asarray(w_init, dtype=np.float32)
    w_qkv = np.asarray(w_qkv, dtype=np.float32)
    w_out = np.asarray(w_out, dtype=np.float32)

    xpads = []
    for b in range(2):
        xp = np.pad(x[b], ((0, 0), (1, 1), (1, 1), (1, 1)), mode="wrap")
        xpads.append(np.ascontiguousarray(xp.reshape(IN_CH, PADV)))

    in_maps = []
    scale = DKH ** -0.5
    for i in range(8):
        b, h = i // 4, i % 4
        W = np.concatenate([
            w_qkv[8 * h:8 * h + 8] * scale,
            w_qkv[32 + 8 * h:32 + 8 * h + 8],
            w_qkv[64 + 8 * h:64 + 8 * h + 8],
            w_init[8 * h:8 * h + 8],
        ], axis=0)  # (32 oc, 32 ic, 3, 3, 3)
        wcv = np.zeros((96, 9, 32), dtype=np.float32)
        for g in range(3):
            for dz in range(3):
                for dy in range(3):
                    wcv[32 * g:32 * g + 32, 3 * dz + dy, :] = W[:, :, dz, dy, g].T
        wout_h = np.ascontiguousarray(w_out[:, 8 * h:8 * h + 8, 0, 0, 0].T)
        sel = np.zeros((128, 16), dtype=np.float32)
        for c in range(3):
            for v in range(9):
                sel[32 * c + v, v] = 1.0
        in_maps.append({
            "sel": sel,
            "xpad": xpads[b].astype(ml_dtypes.bfloat16),
            "wcv": np.ascontiguousarray(wcv.reshape(96, 9 * 32)).astype(ml_dtypes.bfloat16),
            "wout": wout_h.astype(np.float32),
        })
    return in_maps


def kernel(x, w_init, w_qkv, w_out, b_out):
    from concourse.bass_utils import run_bass_kernel_spmd

    nc = _get_nc()
    in_maps = _host_inputs(x, w_init, w_qkv, w_out)
    res = run_bass_kernel_spmd(nc, in_maps, core_ids=list(range(8)))

    b_out = np.asarray(b_out, dtype=np.float32)
    out = np.zeros((2, 64, NSP), dtype=np.float32)
    for i in range(8):
        b, h = i // 4, i % 4
        out[b, 8 * h:8 * h + 8] = np.asarray(res.results[i]["iout"], dtype=np.float32)
        out[b, 32:64] += res.results[i]["pout"]
    out[:, 32:64] += b_out[None, :, None]
    return out.reshape(2, 64, S, S, S)


# revision 31
# speedup vs baseline: 1.3536x; 1.0071x over previous
"""AConvCircular3D kernel for 8 trn2 NeuronCores.

Sharding: core i handles (batch b = i//4, head h = i%4).
Per core, for its (b, h):
  - 3x3x3 circular conv of x[b] -> 32 channels [q(8) k(8) v(8) init(8)]
    (q-scale folded into weights; K=96 contraction via 3 dx-shifted
    copies of padded x; f32r matmuls)
  - softmax attention for head h (N=4096, dk=dv=8), no max-subtraction:
      for each n-quarter: for each key-tile t:
        ST = K_t^T Q (f32r)  -> exp on ScalarE -> ET (bf16)
        AV^T[v, n] += V'_t @ ET   (V' = [V; ones] stationary, 9 cols)
      denominator comes from the ones row; normalize after transpose.
  - torch-faithful reshape (n-major A) via DRAM bounce, partial 1x1 conv
Host sums the 1x1 partials over the 4 head-cores per batch, adds bias,
and concatenates with the gathered init channels.
"""
import os
import sys

for _p in ("/opt/trn_rl_repo", "/root/.axon_site/_ro/trn_rl_repo"):
    if os.path.isdir(_p) and _p not in sys.path:
        sys.path.insert(0, _p)

import numpy as np
import ml_dtypes

NUM_HEADS = 4
DKH = 8
DVH = 8
IN_CH = 32
S = 16
NSP = S * S * S           # 4096 spatial positions
PADW = S + 2              # 18
PADV = PADW ** 3          # 5832
M_TILE = 128
N_MTILES = NSP // M_TILE  # 32

_CACHE = {}


LDW_OPT = False


def _patch_ldw_opt():
    import concourse.bass_utils as bu
    if getattr(bu, "_ldw_patched", False):
        return
    orig = bu.run_command

    def run_command_ldw(cmd, *a, **kw):
        if isinstance(cmd, list):
            cmd = ["--enable-ldw-opt=true" if c == "--enable-ldw-opt=false" else c
                   for c in cmd]
        return orig(cmd, *a, **kw)

    bu.run_command = run_command_ldw
    bu._ldw_patched = True


def _build(n_cores=8):
    import concourse.bass as bass
    import concourse.mybir as mybir
    import concourse.tile as tile
    from concourse.tile import add_dep_helper
    from concourse import bacc
    from concourse.bass import ts
    from concourse.masks import make_identity

    BF16 = mybir.dt.bfloat16
    F32 = mybir.dt.float32
    F32R = mybir.dt.float32r
    EXP = mybir.ActivationFunctionType.Exp
    COPY = mybir.ActivationFunctionType.Copy

    if LDW_OPT:
        _patch_ldw_opt()
    nc = bacc.Bacc("TRN2", target_bir_lowering=False, debug=False,
                   num_devices=n_cores)

    xpad = nc.declare_dram_parameter("xpad", [IN_CH, PADV], BF16, isOutput=False)
    wcv = nc.declare_dram_parameter("wcv", [96, 9 * 32], BF16, isOutput=False)
    wout = nc.declare_dram_parameter("wout", [8, 32], F32, isOutput=False)
    selp = nc.declare_dram_parameter("sel", [128, 16], F32, isOutput=False)
    iout = nc.declare_dram_parameter("iout", [8, NSP], BF16, isOutput=True)
    pout = nc.declare_dram_parameter("pout", [32, NSP], F32, isOutput=True)

    with tile.TileContext(nc) as tc:
        with tc.tile_pool(name="sb", bufs=1) as sb, \
             tc.tile_pool(name="et", bufs=4) as etp, \
             tc.tile_pool(name="dr", bufs=1, space="DRAM") as drp:
            a_bounce = drp.tile([NSP, DVH], F32)

            # ---- stage padded x: 4 overlapping z-slabs, replicated 3x with
            #      dx shifts, so conv chunk c only waits for slab c//2 ----
            ZSL = 6 * PADW * PADW           # slab = 6 z-slices
            slabs = []
            for sl in range(4):
                z0 = 4 * sl
                xs = sb.tile([96, ZSL], BF16, tag=f"xs{sl}")
                base = z0 * PADW * PADW
                for g in range(3):
                    lo = base + g
                    hi = min(lo + ZSL, PADV)
                    eng = nc.sync if (sl + g) % 2 == 0 else nc.gpsimd
                    eng.dma_start(out=xs[32 * g:32 * g + 32, 0:hi - lo],
                                  in_=xpad[:, lo:hi])
                slabs.append(xs[:].rearrange("p (z y x) -> p z y x",
                                             z=6, y=PADW, x=PADW))

            w_sb = sb.tile([96, 9 * 32], BF16)
            nc.sync.dma_start(out=w_sb[:], in_=wcv[:])
            wout_f = sb.tile([8, 32], F32)
            nc.sync.dma_start(out=wout_f[:], in_=wout[:])
            wout_b = sb.tile([8, 32], BF16)
            nc.vector.tensor_copy(out=wout_b[:], in_=wout_f[:])
            ident = sb.tile([128, 128], BF16)
            make_identity(nc, ident)

            stg16 = sb.tile([32, NSP], BF16)
            q_rep = sb.tile([72, NSP], BF16)
            k_rep = sb.tile([72, NSP], BF16)
            v_sb = sb.tile([9, NSP], BF16)
            nc.vector.memset(v_sb[:], 1.0)
            sel_sb = sb.tile([128, 16], F32)
            nc.sync.dma_start(out=sel_sb[:], in_=selp[:])

            # ---- conv: 9 rounds (dz,dy) x 8 chunks (z pairs), K=96, f32r ----
            # evictions + K/V remap chunked so attention can start early
            with tc.tile_pool(name="cv", bufs=1, space="PSUM") as cvp:
                cv = cvp.tile([32, NSP], F32)
                for cp in range(4):
                    for r in range(9):
                        dz, dy = r // 3, r % 3
                        for c in (2 * cp, 2 * cp + 1):
                            sl = c // 2
                            zb = 2 * c - 4 * sl
                            nc.tensor.matmul(cv[:, ts(c, 512)],
                                             w_sb[:, ts(r, 32)],
                                             slabs[sl][:, zb + dz:zb + dz + 2,
                                                       dy:dy + S, 0:S],
                                             start=(r == 0), stop=(r == 8))
                    w0 = cp * 1024
                    nc.vector.tensor_copy(out=stg16[:, w0:w0 + 1024],
                                          in_=cv[:, w0:w0 + 1024])
                    for r in range(3):
                        nc.sync.dma_start(
                            out=q_rep[32 * r:32 * r + 8, w0:w0 + 1024],
                            in_=stg16[0:8, w0:w0 + 1024])
                        nc.sync.dma_start(
                            out=k_rep[32 * r:32 * r + 8, w0:w0 + 1024],
                            in_=stg16[8:16, w0:w0 + 1024])
                    nc.gpsimd.dma_start(out=v_sb[0:8, w0:w0 + 1024],
                                        in_=stg16[16:24, w0:w0 + 1024])
                    nc.gpsimd.dma_start(out=iout[:, w0:w0 + 1024],
                                        in_=stg16[24:32, w0:w0 + 1024])


            # ---- VT' tiles: transpose V' [9,128] chunks -> vt[128, t, 9] ----
            vt = sb.tile([128, N_MTILES, 9], BF16)
            with tc.tile_pool(name="tp", bufs=2, space="PSUM") as tpp:
                for t in range(N_MTILES):
                    tp = tpp.tile([128, 9], F32)
                    nc.tensor.matmul(tp[:], v_sb[:, ts(t, M_TILE)],
                                     ident[0:9, 0:9], start=True, stop=True)
                    nc.vector.tensor_copy(out=vt[:, t, :], in_=tp[:])

            # ---- attention: n-512-chunk outer, m-groups of 3 inner ----
            # 3-way row-tiled logits (bands 0/32/64 of q_rep/k_rep) and
            # 3-way col-tiled AV partials (av bands 0/32/64); the ones row
            # of V' gives the softmax denominator. Band partials are summed
            # (and transposed to [n, 9]) by a selector matmul afterwards.
            groups = [(3 * g, min(3, N_MTILES - 3 * g))
                      for g in range((N_MTILES + 2) // 3)]
            av_cat = sb.tile([128, 8, 512], F32)
            zc = sb.tile([1, 128], BF16)
            zr = sb.tile([1, 512], BF16)
            nc.vector.memset(zc[:], 0.0)
            nc.vector.memset(zr[:], 0.0)
            with tc.tile_pool(name="av", bufs=2, space="PSUM") as avp, \
                 tc.tile_pool(name="lg", bufs=2, space="PSUM") as lgp:
                for ch in range(8):
                    av_ch = avp.tile([128, 512], F32)
                    # zero the whole accumulator (incl. unused partition rows,
                    # which would otherwise contain stale psum NaNs) and set
                    # has_written bank-wide; creates ordering dep for all bands
                    nc.tensor.matmul(av_ch[:], zc[:], zr[:], start=True,
                                     stop=False)
                    for gi, (t0, gn) in enumerate(groups):
                        lg = lgp.tile([128, 512 * gn], F32, tag="lg")
                        for r in range(gn):
                            nc.tensor.matmul(
                                lg[:, ts(r, 512)],
                                k_rep[32 * r:32 * r + 8, ts(t0 + r, M_TILE)],
                                q_rep[32 * r:32 * r + 8, ts(ch, 512)],
                                start=True, stop=True,
                                tile_position=(32 * r, 0))
                        et = etp.tile([128, 512 * gn], BF16, tag="et")
                        nc.scalar.activation(out=et[:], in_=lg[:], func=EXP)
                        for r in range(gn):
                            nc.tensor.matmul(
                                av_ch[32 * r:32 * r + 9, :],
                                vt[:, t0 + r, :],
                                et[:, ts(r, 512)],
                                start=False, stop=(gi == len(groups) - 1),
                                tile_position=(0, 32 * r))
                    nc.vector.tensor_copy(out=av_cat[:, ch, :], in_=av_ch[:])

            # ---- band-reduce + transpose via selector matmul ----
            # avf[n128, v] = sum_p av_cat[p, n] * sel[p, v]
            avf = sb.tile([128, N_MTILES, 9], F32)
            with tc.tile_pool(name="tq", bufs=2, space="PSUM") as tqp:
                for t in range(N_MTILES):
                    ch, off = t // 4, (t % 4) * 128
                    tq = tqp.tile([128, 9], F32)
                    nc.tensor.matmul(tq[:],
                                     av_cat[:, ch, off:off + M_TILE],
                                     sel_sb[:, 0:9], start=True, stop=True)
                    nc.vector.tensor_copy(out=avf[:, t, :], in_=tq[:])
            rcp = sb.tile([128, N_MTILES], F32)
            nc.vector.reciprocal(out=rcp[:], in_=avf[:, :, 8])
            a_sb = sb.tile([128, N_MTILES, 8], F32)
            nc.vector.tensor_mul(a_sb[:], avf[:, :, 0:8],
                                 rcp[:].to_broadcast([128, N_MTILES, 8]))

            # ---- funky reshape via DRAM bounce (f32: dma reads of high
            #      bf16 sbuf tiles at 128 partitions corrupt p>=64) ----
            # split into halves over the low n-bits so comb half 0 and the
            # first 1x1 matmuls can start while half 1 is still in flight
            ab5 = a_bounce[:].rearrange("(a b c p) v -> p a b c v",
                                         a=8, b=2, c=2, p=128)
            a5 = a_sb[:].rearrange("p (a b c) v -> p a b c v", b=2, c=2)
            comb = sb.tile([8, NSP], BF16)
            rdmas = []
            for hh in range(2):
                wds = [nc.sync.dma_start(out=ab5[:, :, hh, cc, :],
                                         in_=a5[:, :, hh, cc, :])
                       for cc in range(2)]
                rd = nc.gpsimd.dma_start(
                    out=comb[:, ts(hh, 2048)],
                    in_=a_bounce[:].rearrange("(c hh2 r) v -> c hh2 (r v)",
                                              c=8, hh2=2)[:, hh, :])
                for wd in wds:
                    add_dep_helper(rd.ins, wd.ins,
                                   reason="a_bounce dram RAW half")
                rdmas.append(rd)

            # ---- partial 1x1 conv ----
            po_sb = sb.tile([32, NSP], F32)
            with tc.tile_pool(name="po", bufs=1, space="PSUM") as pop:
                po = pop.tile([32, NSP], F32)
                for c in range(8):
                    nc.tensor.matmul(po[:, ts(c, 512)], wout_b[:],
                                     comb[:, ts(c, 512)], start=True, stop=True)
                    if c % 2 == 0:
                        nc.scalar.activation(out=po_sb[:, ts(c, 512)],
                                             in_=po[:, ts(c, 512)], func=COPY)
                    else:
                        nc.vector.tensor_copy(out=po_sb[:, ts(c, 512)],
                                              in_=po[:, ts(c, 512)])
                    nc.sync.dma_start(out=pout[:, ts(c, 512)],
                                      in_=po_sb[:, ts(c, 512)])

    nc.compile()
    return nc


def _get_nc(n_cores=8):
    if n_cores not in _CACHE:
        _CACHE[n_cores] = _build(n_cores)
    return _CACHE[n_cores]


def _host_inputs(x, w_init, w_qkv, w_out):
    """Build per-core input maps. Core i = (b = i//4, h = i%4)."""
    x = np.asarray(x, dtype=np.float32)
    w_init = np.asarray(w_init, dtype=np.float32)
    w_qkv = np.asarray(w_qkv, dtype=np.float32)
    w_out = np.asarray(w_out, dtype=np.float32)

    xpads = []
    for b in range(2):
        xp = np.pad(x[b], ((0, 0), (1, 1), (1, 1), (1, 1)), mode="wrap")
        xpads.append(np.ascontiguousarray(xp.reshape(IN_CH, PADV)))

    in_maps = []
    scale = DKH ** -0.5
    for i in range(8):
        b, h = i // 4, i % 4
        W = np.concatenate([
            w_qkv[8 * h:8 * h + 8] * scale,
            w_qkv[32 + 8 * h:32 + 8 * h + 8],
            w_qkv[64 + 8 * h:64 + 8 * h + 8],
            w_init[8 * h:8 * h + 8],
        ], axis=0)  # (32 oc, 32 ic, 3, 3, 3)
        wcv = np.zeros((96, 9, 32), dtype=np.float32)
        for g in range(3):
            for dz in range(3):
                for dy in range(3):
                    wcv[32 * g:32 * g + 32, 3 * dz + dy, :] = W[:, :, dz, dy, g].T
        wout_h = np.ascontiguousarray(w_out[:, 8 * h:8 * h + 8, 0, 0, 0].T)
        sel = np.zeros((128, 16), dtype=np.float32)
        for c in range(3):
            for v in range(9):
                sel[32 * c + v, v] = 1.0
        in_maps.append({
            "sel": sel,
            "xpad": xpads[b].astype(ml_dtypes.bfloat16),
            "wcv": np.ascontiguousarray(wcv.reshape(96, 9 * 32)).astype(ml_dtypes.bfloat16),
            "wout": wout_h.astype(np.float32),
        })
    return in_maps


def kernel(x, w_init, w_qkv, w_out, b_out):
    from concourse.bass_utils import run_bass_kernel_spmd

    nc = _get_nc()
    in_maps = _host_inputs(x, w_init, w_qkv, w_out)
    res = run_bass_kernel_spmd(nc, in_maps, core_ids=list(range(8)))

    b_out = np.asarray(b_out, dtype=np.float32)
    out = np.zeros((2, 64, NSP), dtype=np.float32)
    for i in range(8):
        b, h = i // 4, i % 4
        out[b, 8 * h:8 * h + 8] = np.asarray(res.results[i]["iout"], dtype=np.float32)
        out[b, 32:64] += res.results[i]["pout"]
    out[:, 32:64] += b_out[None, :, None]
    return out.reshape(2, 64, S, S, S)
